# revision 14
# baseline (speedup 1.0000x reference)
"""DMoN GCN (3-layer) Trainium2 kernel over 8 NeuronCores — single SPMD launch.

Sharding: core c of 8 handles edges with source in pair-slice s=c//2 (25088
padded nodes, int16-indexable) and target owned by cores of parity g=c%2.
Per layer: PE transform -> pair AllGather of the bf16 node table (split in 2
halves, pipelined behind the transform) -> dma_gather of source rows across
all 4 SWDGE queues + PE matmuls against host-shipped weighted one-hot masks,
accumulating exact segment sums in PSUM (duplicate-safe, no scatter) ->
4 region ReduceScatters (bf16, emitted 3 chunks late so their waits are
pre-satisfied) -> combine + selu (softmax at the end). gcn_norm weights are
baked into the masks on the host; self-loop messages ride the skip path
(dd = dinv^2 per node, applied to the transformed features in PSUM).
"""

import numpy as np
import ml_dtypes

BF16 = ml_dtypes.bfloat16

N = 100000
IN_DIM = 256
HID = 128
HID2 = 64
K = 16

NCORE = 8
NOWN = 12544            # 98*128 padded nodes per core
NPAIR = 2 * NOWN        # 25088 source rows per pair  (< 32768 -> int16)
NTGT = 4 * NOWN         # 50176 targets per parity group
NBLK = NTGT // 128      # 392 target blocks
CHUNK = 8192            # slots per dma_gather
SUB = CHUNK // 128

# target regions (4 sub-ReduceScatters per layer), in own-row units
K_START = [0, 3200, 6400, 9600]
NK = [3200, 3200, 3200, 2944]
B_K = [0, 12800, 25600, 38400]          # d_part base row per region
RBLK_EDGE = [0, 100, 200, 300, 392]     # aggregation block edges per region
NTIL = [25, 25, 25, 23]                 # 128-row tiles per region
# source halves (2 sub-AllGathers per layer)
H_START = [0, 6400]
NH = [6400, 6144]
HB = [0, 12800]                         # pair-table base row per half

SELU_L = 1.0507009873554805
SELU_A = 1.6732632423543772

_CACHE = {}


def _pad_id(r):
    c = r // 12500
    return c * NOWN + (r - c * 12500)


def _build_plan(edge_index, edge_weight):
    row = np.asarray(edge_index[0], dtype=np.int64)
    col = np.asarray(edge_index[1], dtype=np.int64)
    w = np.asarray(edge_weight, dtype=np.float64)

    deg = np.zeros(N, np.float64)
    np.add.at(deg, col, w)
    deg += 1.0
    dinv = np.where(deg > 0, 1.0 / np.sqrt(deg), 0.0)
    nw = dinv[row] * w * dinv[col]
    dd = dinv * dinv              # self-loop weight -> skip path

    rp = _pad_id(row)
    cp = _pad_id(col)

    npH = np.array(NH)
    npHB = np.array(HB)
    npHS = np.array(H_START)
    s_core = rp // NOWN
    s_oo = rp - s_core * NOWN
    s_h = (s_oo >= 6400).astype(np.int64)
    src_pair = s_core // 2
    er_pos = npHB[s_h] + (s_core % 2) * npH[s_h] + (s_oo - npHS[s_h])

    npNK = np.array(NK)
    npBK = np.array(B_K)
    npKS = np.array(K_START)
    t_core = cp // NOWN
    t_oo = cp - t_core * NOWN
    t_k = np.minimum(t_oo // 3200, 3)
    tgt_par = t_core % 2
    agg_pos = npBK[t_k] + (t_core // 2) * npNK[t_k] + (t_oo - npKS[t_k])

    per_core = []
    for c in range(NCORE):
        s, g = c // 2, c % 2
        sel = (src_pair == s) & (tgt_par == g)
        er = er_pos[sel]
        ec = agg_pos[sel]
        ew = nw[sel]
        o = np.argsort(ec, kind="stable")
        er, ec, ew = er[o], ec[o], ew[o]
        blk = ec // 128
        cnt = np.bincount(blk, minlength=NBLK)
        pcnt = np.maximum(((cnt + 127) // 128) * 128, 128)
        per_core.append((er, ec, ew, blk, cnt, pcnt))

    # shared program structure: per-block sub-chunk counts = max across cores
    nsub_blk = np.stack([p[5] // 128 for p in per_core]).max(axis=0)
    tot_sub = int(nsub_blk.sum())
    nchunk = (tot_sub + SUB - 1) // SUB
    pad_sub = nchunk * SUB - tot_sub
    nsub_blk = nsub_blk.copy()
    nsub_blk[-1] += pad_sub
    tot_sub = nchunk * SUB
    sub_off = np.concatenate([[0], np.cumsum(nsub_blk)])[:-1]
    blk_of_sub = np.repeat(np.arange(NBLK), nsub_blk)
    start_of_sub = np.zeros(tot_sub, bool)
    start_of_sub[sub_off] = True

    idx_all, msk_all = [], []
    ii = np.arange(CHUNK)
    for c in range(NCORE):
        er, ec, ew, blk, cnt, _ = per_core[c]
        nslot = tot_sub * 128
        sidx = np.zeros(nslot, np.int16)
        stgt = np.zeros(nslot, np.int64)
        sw = np.zeros(nslot)
        cnt_off = np.concatenate([[0], np.cumsum(cnt)])[:-1]
        pos = sub_off[blk] * 128 + (np.arange(len(ec)) - cnt_off[blk])
        sidx[pos] = er.astype(np.int16)
        stgt[pos] = ec - blk * 128
        sw[pos] = ew
        idx_w = np.zeros((nchunk, 128, CHUNK // 16), np.int16)
        for ch in range(nchunk):
            seg = sidx[ch * CHUNK:(ch + 1) * CHUNK]
            t16 = np.zeros((16, CHUNK // 16), np.int16)
            t16[ii % 16, ii // 16] = seg
            idx_w[ch] = np.tile(t16, (8, 1))
        masks = np.zeros((tot_sub, 128, 128), np.float32)
        sl = np.arange(nslot)
        masks[sl // 128, sl % 128, stgt] = sw
        # [nchunk, 128(slot), SUB, 128(tgt)] so the per-chunk mask DMA is
        # contiguous along each partition line
        masks = masks.reshape(nchunk, SUB, 128, 128).transpose(0, 2, 1, 3)
        idx_all.append(idx_w)
        msk_all.append(np.ascontiguousarray(masks).astype(BF16))

    NT = NOWN // 128
    ddp = np.zeros((NCORE, 128, NT), np.float32)
    for c in range(NCORE):
        v = np.zeros(NOWN)
        v[:12500] = dd[c * 12500:(c + 1) * 12500]
        ddp[c] = v.reshape(NT, 128).T

    return dict(nchunk=nchunk, blk_of_sub=blk_of_sub,
                start_of_sub=start_of_sub, idx=idx_all, msk=msk_all,
                ddp=ddp.astype(np.float32))


def _build_program(nchunk, blk_of_sub, start_of_sub):
    import concourse.bacc as bacc
    import concourse.mybir as mybir
    import concourse.hw_specs as hw_specs
    from concourse import tile

    # The stock cost model underestimates dma_gather descriptor generation
    # ~17x (calibrated for plain SWDGE dma_start). With the real per-index
    # rate the Tile scheduler stops hoisting combine/selu work into the
    # middle of the gather-bound aggregation (which head-of-line-blocks the
    # in-order DVE/ACT queues and stalls the PSUM eviction pipeline).
    hw_specs.TRN2Spec.SWDGE_NS_PER_DESCRIPTOR = 7.7

    nc = bacc.Bacc("TRN2", target_bir_lowering=False, debug=False,
                   num_devices=NCORE, num_swdge_queues=4,
                   dynamic_dma_scratch_size=32768)
    f32, bf16, i16 = mybir.dt.float32, mybir.dt.bfloat16, mybir.dt.int16
    AL = mybir.AluOpType
    TOT_SUB = nchunk * SUB
    FD = [HID, HID2, K]
    NT = NOWN // 128
    AG_GROUPS = [[0, 1], [2, 3], [4, 5], [6, 7]]
    RS_GROUPS = [[0, 2, 4, 6], [1, 3, 5, 7]]

    t_x = nc.dram_tensor("x", [NOWN, IN_DIM], f32, kind="ExternalInput")
    t_idx = nc.dram_tensor("idx", [nchunk, 128, CHUNK // 16], i16,
                           kind="ExternalInput")
    t_msk = nc.dram_tensor("msk", [nchunk, 128, SUB, 128], bf16,
                           kind="ExternalInput")
    t_W0 = nc.dram_tensor("W0p", [2, 128, HID], f32, kind="ExternalInput")
    t_P0 = nc.dram_tensor("P0p", [2, 128, HID], f32, kind="ExternalInput")
    t_W1 = nc.dram_tensor("W1p", [1, 128, HID2], f32, kind="ExternalInput")
    t_P1 = nc.dram_tensor("P1p", [1, 128, HID2], f32, kind="ExternalInput")
    t_W2 = nc.dram_tensor("W2p", [1, 128, K], f32, kind="ExternalInput")
    t_b0 = nc.dram_tensor("b0r", [128, HID], f32, kind="ExternalInput")
    t_b1 = nc.dram_tensor("b1r", [128, HID2], f32, kind="ExternalInput")
    t_b2 = nc.dram_tensor("b2r", [128, K], f32, kind="ExternalInput")
    t_dd = nc.dram_tensor("ddr", [128, NT], f32, kind="ExternalInput")
    t_id = nc.dram_tensor("ident", [128, 128], f32, kind="ExternalInput")
    t_out = nc.dram_tensor("out", [NOWN, K], f32, kind="ExternalOutput")

    with tile.TileContext(nc) as tc:
        with (
            tc.tile_pool(name="dram", bufs=1, space="DRAM") as dpool,
            tc.tile_pool(name="wts", bufs=1) as wpool,
            tc.tile_pool(name="gbuf", bufs=5) as gpool,
            tc.tile_pool(name="mbuf", bufs=4) as mpool,
            tc.tile_pool(name="work", bufs=4) as pool,
            tc.tile_pool(name="ps", bufs=2, space="PSUM") as ppool,
            tc.tile_pool(name="acc", bufs=2, space="PSUM") as apool,
        ):
            d_tab_own = [[dpool.tile([NH[h], 128], bf16,
                                     tag=f"tabown{l}_{h}",
                                     name=f"tabown{l}_{h}")
                          for h in range(2)] for l in range(3)]
            d_tab_pair = [dpool.tile([NPAIR, 128], bf16, tag=f"tabpair{l}",
                                     name=f"tabpair{l}") for l in range(3)]
            d_sk = [[dpool.tile([NK[k] , FD[l]], f32, tag=f"sk{l}_{k}",
                                name=f"sk{l}_{k}") for k in range(4)]
                    for l in range(3)]
            d_part = [[dpool.tile([4 * NK[k], FD[l]], bf16,
                                  tag=f"part{l}_{k}", name=f"part{l}_{k}")
                       for k in range(4)] for l in range(3)]
            d_rs = [[dpool.tile([NK[k], FD[l]], bf16, tag=f"rs{l}_{k}",
                                name=f"rs{l}_{k}") for k in range(4)]
                    for l in range(3)]
            d_h = [[dpool.tile([NK[k], FD[l]], f32, tag=f"h{l}_{k}",
                               name=f"h{l}_{k}") for k in range(4)]
                   for l in range(2)]

            idt = wpool.tile([128, 128], f32)
            nc.sync.dma_start(idt[:], t_id[:])
            def wload(tname, src_t, n, fdim):
                ts = []
                for j in range(n):
                    wt = wpool.tile([128, fdim], f32, name=f"{tname}_{j}")
                    nc.sync.dma_start(wt[:], src_t[j])
                    ts.append(wt)
                return ts
            wW0 = wload("wW0", t_W0, 2, HID)
            wP0 = wload("wP0", t_P0, 2, HID)
            wW1 = wload("wW1", t_W1, 1, HID2)
            wP1 = wload("wP1", t_P1, 1, HID2)
            wW2 = wload("wW2", t_W2, 1, K)
            wb0 = wpool.tile([128, HID], f32)
            nc.sync.dma_start(wb0[:], t_b0[:])
            wb1 = wpool.tile([128, HID2], f32)
            nc.sync.dma_start(wb1[:], t_b1[:])
            wb2 = wpool.tile([128, K], f32)
            nc.sync.dma_start(wb2[:], t_b2[:])
            wdd = wpool.tile([128, NT], f32)
            nc.sync.dma_start(wdd[:], t_dd[:])

            def transform_tile(l, t, src_ap, fin, fout, Wt, Pt, bias):
                """One 128-row tile: table entry + skip(+self) entry."""
                ncin = (fin + 127) // 128
                k = min(t // 25, 3)
                h = 0 if t < 50 else 1
                xin = pool.tile([128, ncin * 128], f32, tag="xin")
                if fin % 128:
                    nc.vector.memzero(xin[:, fin:])
                nc.sync.dma_start(xin[:, :fin], src_ap)
                xT = pool.tile([128, ncin * 128], f32, tag="xT")
                for j in range(ncin):
                    pt = ppool.tile([128, 128], f32, tag="ptr")
                    nc.tensor.transpose(
                        pt[:], xin[:, j * 128:(j + 1) * 128], idt[:])
                    nc.vector.tensor_copy(
                        xT[:, j * 128:(j + 1) * 128], pt[:])
                pm = ppool.tile([128, fout], f32, tag="pmm")
                for j in range(ncin):
                    nc.tensor.matmul(pm[:], xT[:, j * 128:(j + 1) * 128],
                                     Wt[j][:], start=(j == 0),
                                     stop=(j == ncin - 1))
                tb = pool.tile([128, 128], bf16, tag="tabtile")
                nc.vector.tensor_copy(tb[:, :fout], pm[:])
                nc.scalar.dma_start(
                    d_tab_own[l][h][t * 128 - 6400 * h:
                                    t * 128 - 6400 * h + 128, :], tb[:])
                # skip path, with the self-loop message dd*(h@W) folded in
                sk = pool.tile([128, fout], f32, tag="sktile")
                nc.vector.tensor_scalar(sk[:], pm[:], wdd[:, t:t + 1], None,
                                        AL.mult)
                if Pt is not None:
                    ps = ppool.tile([128, fout], f32, tag="psk")
                    for j in range(ncin):
                        nc.tensor.matmul(ps[:], xT[:, j * 128:(j + 1) * 128],
                                         Pt[j][:], start=(j == 0),
                                         stop=(j == ncin - 1))
                    nc.vector.tensor_tensor(sk[:], sk[:], ps[:], AL.add)
                nc.vector.tensor_tensor(sk[:], sk[:], bias[:, :fout], AL.add)
                nc.scalar.dma_start(
                    d_sk[l][k][t * 128 - 3200 * k:
                               t * 128 - 3200 * k + 128, :], sk[:])

            def allgather(l, h):
                nc.gpsimd.collective_compute(
                    "AllGather", mybir.AluOpType.bypass,
                    replica_groups=AG_GROUPS,
                    ins=[d_tab_own[l][h][:].opt()],
                    outs=[d_tab_pair[l][HB[h]:HB[h] + 2 * NH[h], :].opt()])

            def aggregate(l, fout):
                acc = None
                sub = 0
                rs_pending = []
                def emit_rs(k):
                    nc.gpsimd.collective_compute(
                        "ReduceScatter", mybir.AluOpType.add,
                        replica_groups=RS_GROUPS,
                        ins=[d_part[l][k][:].opt()],
                        outs=[d_rs[l][k][:].opt()])
                for ch in range(nchunk):
                    while rs_pending and ch >= rs_pending[0][1]:
                        emit_rs(rs_pending.pop(0)[0])
                    it = pool.tile([128, CHUNK // 16], i16, tag="idxt",
                                   bufs=5)
                    nc.sync.dma_start(it[:], t_idx[ch])
                    g = gpool.tile([128, SUB, 128], bf16, tag="gath")
                    nc.gpsimd.dma_gather(g[:], d_tab_pair[l][:], it[:],
                                         CHUNK, CHUNK, 128,
                                         single_packet=False,
                                         queue_num=ch % 4)
                    mk = mpool.tile([128, SUB, 128], bf16, tag="maskt")
                    nc.sync.dma_start(mk[:], t_msk[ch])
                    for j in range(SUB):
                        st = bool(start_of_sub[sub])
                        if st:
                            acc = apool.tile([128, fout], f32, tag="accps",
                                             name=f"acc{l}_{sub}")
                        last = (sub == TOT_SUB - 1) or bool(
                            start_of_sub[sub + 1])
                        nc.tensor.matmul(acc[:], mk[:, j, :],
                                         g[:, j, :fout], start=st, stop=last)
                        if last:
                            blk = int(blk_of_sub[sub])
                            k = min(blk // 100, 3)
                            lofs = (blk - RBLK_EDGE[k]) * 128
                            ev = pool.tile([128, fout], bf16, tag="ev", bufs=8)
                            nc.vector.tensor_copy(ev[:], acc[:])
                            nc.scalar.dma_start(
                                d_part[l][k][lofs:lofs + 128, :], ev[:])
                            if blk == RBLK_EDGE[k + 1] - 1:
                                rs_pending.append((k, ch + 3))
                        sub += 1
                for k, _ in rs_pending:
                    emit_rs(k)

            def combine(l, fout, k):
                for tt in range(NTIL[k]):
                    zb = pool.tile([128, fout], bf16, tag="zb", bufs=8)
                    nc.sync.dma_start(zb[:],
                                      d_rs[l][k][tt * 128:(tt + 1) * 128, :])
                    sk = pool.tile([128, fout], f32, tag="skld", bufs=8)
                    nc.sync.dma_start(sk[:],
                                      d_sk[l][k][tt * 128:(tt + 1) * 128, :])
                    zz = pool.tile([128, fout], f32, tag="z", bufs=8)
                    nc.vector.tensor_tensor(zz[:], zb[:], sk[:], AL.add)
                    mn = pool.tile([128, fout], f32, tag="smn", bufs=8)
                    nc.vector.tensor_scalar_min(mn[:], zz[:], 0.0)
                    ex = pool.tile([128, fout], f32, tag="sex", bufs=8)
                    nc.scalar.activation(ex[:], mn[:],
                                         mybir.ActivationFunctionType.Exp)
                    nc.vector.tensor_scalar(ex[:], ex[:], SELU_L * SELU_A,
                                            -SELU_L * SELU_A, AL.mult,
                                            AL.add)
                    nc.vector.tensor_scalar_max(zz[:], zz[:], 0.0)
                    nc.vector.tensor_scalar(zz[:], zz[:], SELU_L, None,
                                            AL.mult)
                    nc.vector.tensor_tensor(zz[:], zz[:], ex[:], AL.add)
                    if l < 2:
                        nc.scalar.dma_start(
                            d_h[l][k][tt * 128:(tt + 1) * 128, :], zz[:])
                    else:
                        mx = pool.tile([128, 1], f32, tag="smx")
                        nc.vector.tensor_reduce(mx[:], zz[:],
                                                mybir.AxisListType.X, AL.max)
                        nc.vector.tensor_scalar(zz[:], zz[:], mx[:], None,
                                                AL.subtract)
                        nc.scalar.activation(
                            zz[:], zz[:], mybir.ActivationFunctionType.Exp)
                        sm = pool.tile([128, 1], f32, tag="ssm")
                        nc.vector.tensor_reduce(sm[:], zz[:],
                                                mybir.AxisListType.X, AL.add)
                        rc = pool.tile([128, 1], f32, tag="src")
                        nc.vector.reciprocal(rc[:], sm[:])
                        nc.vector.tensor_scalar(zz[:], zz[:], rc[:], None,
                                                AL.mult)
                        gofs = K_START[k] + tt * 128
                        nc.scalar.dma_start(t_out[gofs:gofs + 128, :], zz[:])

            fins = [IN_DIM, HID, HID2]
            Ws = [wW0, wW1, wW2]
            Ps = [wP0, wP1, None]
            bs = [wb0, wb1, wb2]

            # layer-0 transform over all tiles, AGs pipelined per half
            for t in range(NT):
                transform_tile(0, t, t_x[t * 128:(t + 1) * 128, :],
                               fins[0], FD[0], Ws[0], Ps[0], bs[0])
                if t == 49:
                    allgather(0, 0)
            allgather(0, 1)

            for l in range(3):
                aggregate(l, FD[l])
                for k in range(4):
                    combine(l, FD[l], k)
                    if l < 2:
                        l2 = l + 1
                        for t in range(25 * k, 25 * k + NTIL[k]):
                            src = d_h[l][k][t * 128 - 3200 * k:
                                            t * 128 - 3200 * k + 128, :]
                            transform_tile(l2, t, src, fins[l2], FD[l2],
                                           Ws[l2], Ps[l2], bs[l2])
                        if k == 1:
                            allgather(l2, 0)
                        if k == 3:
                            allgather(l2, 1)
    nc.compile()
    return nc


def _get_compiled(inputs):
    k = "prog"
    if k not in _CACHE:
        plan = _build_plan(inputs["edge_index"], inputs["edge_weight"])
        nc = _build_program(plan["nchunk"], plan["blk_of_sub"],
                            plan["start_of_sub"])
        _CACHE[k] = (plan, nc)
    return _CACHE[k]


def kernel(_trace=False, **inputs):
    from concourse.bass_utils import run_bass_kernel_spmd

    plan, nc = _get_compiled(inputs)

    x = np.asarray(inputs["x"], np.float32)
    xpad = np.zeros((NCORE, NOWN, IN_DIM), np.float32)
    for c in range(NCORE):
        xpad[c, :12500] = x[c * 12500:(c + 1) * 12500]

    def wchunks(W, n):
        out = np.zeros((n, 128, W.shape[1]), np.float32)
        for j in range(n):
            out[j, :min(128, W.shape[0] - j * 128)] = \
                W[j * 128:(j + 1) * 128]
        return out

    W0 = np.asarray(inputs["W0"], np.float32)
    P0 = np.asarray(inputs["P0w"], np.float32)
    W1p = wchunks(np.asarray(inputs["W1"], np.float32), 1)
    P1p = wchunks(np.asarray(inputs["P1w"], np.float32), 1)
    W2p = wchunks(np.asarray(inputs["W2"], np.float32), 1)
    b0r = np.tile((np.asarray(inputs["b0"]) + np.asarray(inputs["P0b"]))
                  .astype(np.float32), (128, 1))
    b1r = np.tile((np.asarray(inputs["b1"]) + np.asarray(inputs["P1b"]))
                  .astype(np.float32), (128, 1))
    b2r = np.tile(np.asarray(inputs["b2"]).astype(np.float32), (128, 1))

    in_maps = []
    for c in range(NCORE):
        in_maps.append({
            "x": xpad[c],
            "idx": plan["idx"][c], "msk": plan["msk"][c],
            "W0p": wchunks(W0, 2), "P0p": wchunks(P0, 2),
            "W1p": W1p, "P1p": P1p, "W2p": W2p,
            "b0r": b0r, "b1r": b1r, "b2r": b2r,
            "ddr": plan["ddp"][c],
            "ident": np.eye(128, dtype=np.float32),
        })
    res = run_bass_kernel_spmd(nc, in_maps, core_ids=list(range(NCORE)),
                               trace=_trace)
    if _trace:
        kernel.last_exec_ns = res.exec_time_ns
    out = np.zeros((N, K), np.float32)
    for c in range(NCORE):
        out[c * 12500:(c + 1) * 12500] = res.results[c]["out"][:12500]
    return out


# revision 18
# speedup vs baseline: 1.0631x; 1.0631x over previous
"""DMoN GCN (3-layer) Trainium2 kernel over 8 NeuronCores — single SPMD launch.

Sharding: core c of 8 handles edges with source in pair-slice s=c//2 (25088
padded nodes, int16-indexable) and target owned by cores of parity g=c%2.
Per layer: PE transform -> pair AllGather of the bf16 node table (split in 2
halves, pipelined behind the transform) -> dma_gather of source rows across
all 4 SWDGE queues + PE matmuls against host-shipped weighted one-hot masks,
accumulating exact segment sums in PSUM (duplicate-safe, no scatter) ->
4 region ReduceScatters (bf16, emitted 3 chunks late so their waits are
pre-satisfied) -> combine + selu (softmax at the end). gcn_norm weights are
baked into the masks on the host; self-loop messages ride the skip path
(dd = dinv^2 per node, applied to the transformed features in PSUM).
"""

import numpy as np
import ml_dtypes

BF16 = ml_dtypes.bfloat16

N = 100000
IN_DIM = 256
HID = 128
HID2 = 64
K = 16

NCORE = 8
NOWN = 12544            # 98*128 padded nodes per core
NPAIR = 2 * NOWN        # 25088 source rows per pair  (< 32768 -> int16)
NTGT = 4 * NOWN         # 50176 targets per parity group
NBLK = NTGT // 128      # 392 target blocks
CHUNK = 8192            # slots per dma_gather
SUB = CHUNK // 128

# target regions (4 sub-ReduceScatters per layer), in own-row units
K_START = [0, 3200, 6400, 9600]
NK = [3200, 3200, 3200, 2944]
B_K = [0, 12800, 25600, 38400]          # d_part base row per region
RBLK_EDGE = [0, 100, 200, 300, 392]     # aggregation block edges per region
NTIL = [25, 25, 25, 23]                 # 128-row tiles per region
# source halves (2 sub-AllGathers per layer)
H_START = [0, 6400]
NH = [6400, 6144]
HB = [0, 12800]                         # pair-table base row per half

SELU_L = 1.0507009873554805
SELU_A = 1.6732632423543772

_CACHE = {}


def _pad_id(r):
    c = r // 12500
    return c * NOWN + (r - c * 12500)


def _build_plan(edge_index, edge_weight):
    row = np.asarray(edge_index[0], dtype=np.int64)
    col = np.asarray(edge_index[1], dtype=np.int64)
    w = np.asarray(edge_weight, dtype=np.float64)

    deg = np.zeros(N, np.float64)
    np.add.at(deg, col, w)
    deg += 1.0
    dinv = np.where(deg > 0, 1.0 / np.sqrt(deg), 0.0)
    nw = dinv[row] * w * dinv[col]
    dd = dinv * dinv              # self-loop weight -> skip path

    rp = _pad_id(row)
    cp = _pad_id(col)

    npH = np.array(NH)
    npHB = np.array(HB)
    npHS = np.array(H_START)
    s_core = rp // NOWN
    s_oo = rp - s_core * NOWN
    s_h = (s_oo >= 6400).astype(np.int64)
    src_pair = s_core // 2
    er_pos = npHB[s_h] + (s_core % 2) * npH[s_h] + (s_oo - npHS[s_h])

    npNK = np.array(NK)
    npBK = np.array(B_K)
    npKS = np.array(K_START)
    t_core = cp // NOWN
    t_oo = cp - t_core * NOWN
    t_k = np.minimum(t_oo // 3200, 3)
    tgt_par = t_core % 2
    agg_pos = npBK[t_k] + (t_core // 2) * npNK[t_k] + (t_oo - npKS[t_k])

    per_core = []
    for c in range(NCORE):
        s, g = c // 2, c % 2
        sel = (src_pair == s) & (tgt_par == g)
        er = er_pos[sel]
        ec = agg_pos[sel]
        ew = nw[sel]
        o = np.argsort(ec, kind="stable")
        er, ec, ew = er[o], ec[o], ew[o]
        blk = ec // 128
        cnt = np.bincount(blk, minlength=NBLK)
        pcnt = np.maximum(((cnt + 127) // 128) * 128, 128)
        per_core.append((er, ec, ew, blk, cnt, pcnt))

    # shared program structure: per-block sub-chunk counts = max across cores
    nsub_blk = np.stack([p[5] // 128 for p in per_core]).max(axis=0)
    tot_sub = int(nsub_blk.sum())
    nchunk = (tot_sub + SUB - 1) // SUB
    pad_sub = nchunk * SUB - tot_sub
    nsub_blk = nsub_blk.copy()
    nsub_blk[-1] += pad_sub
    tot_sub = nchunk * SUB
    sub_off = np.concatenate([[0], np.cumsum(nsub_blk)])[:-1]
    blk_of_sub = np.repeat(np.arange(NBLK), nsub_blk)
    start_of_sub = np.zeros(tot_sub, bool)
    start_of_sub[sub_off] = True

    idx_all, msk_all = [], []
    ii = np.arange(CHUNK)
    for c in range(NCORE):
        er, ec, ew, blk, cnt, _ = per_core[c]
        nslot = tot_sub * 128
        sidx = np.zeros(nslot, np.int16)
        stgt = np.zeros(nslot, np.int64)
        sw = np.zeros(nslot)
        cnt_off = np.concatenate([[0], np.cumsum(cnt)])[:-1]
        pos = sub_off[blk] * 128 + (np.arange(len(ec)) - cnt_off[blk])
        sidx[pos] = er.astype(np.int16)
        stgt[pos] = ec - blk * 128
        sw[pos] = ew
        idx_w = np.zeros((nchunk, 128, CHUNK // 16), np.int16)
        for ch in range(nchunk):
            seg = sidx[ch * CHUNK:(ch + 1) * CHUNK]
            t16 = np.zeros((16, CHUNK // 16), np.int16)
            t16[ii % 16, ii // 16] = seg
            idx_w[ch] = np.tile(t16, (8, 1))
        masks = np.zeros((tot_sub, 128, 128), np.float32)
        sl = np.arange(nslot)
        masks[sl // 128, sl % 128, stgt] = sw
        # [nchunk, 128(slot), SUB, 128(tgt)] so the per-chunk mask DMA is
        # contiguous along each partition line
        masks = masks.reshape(nchunk, SUB, 128, 128).transpose(0, 2, 1, 3)
        idx_all.append(idx_w)
        msk_all.append(np.ascontiguousarray(masks).astype(BF16))

    NT = NOWN // 128
    ddp = np.zeros((NCORE, 128, NT), np.float32)
    for c in range(NCORE):
        v = np.zeros(NOWN)
        v[:12500] = dd[c * 12500:(c + 1) * 12500]
        ddp[c] = v.reshape(NT, 128).T

    return dict(nchunk=nchunk, blk_of_sub=blk_of_sub,
                start_of_sub=start_of_sub, idx=idx_all, msk=msk_all,
                ddp=ddp.astype(np.float32))


def _build_program(nchunk, blk_of_sub, start_of_sub):
    import concourse.bacc as bacc
    import concourse.mybir as mybir
    import concourse.hw_specs as hw_specs
    from concourse import tile

    # The stock cost model underestimates dma_gather descriptor generation
    # ~17x (calibrated for plain SWDGE dma_start). With the real per-index
    # rate the Tile scheduler stops hoisting combine/selu work into the
    # middle of the gather-bound aggregation (which head-of-line-blocks the
    # in-order DVE/ACT queues and stalls the PSUM eviction pipeline).
    hw_specs.TRN2Spec.SWDGE_NS_PER_DESCRIPTOR = 7.7

    nc = bacc.Bacc("TRN2", target_bir_lowering=False, debug=False,
                   num_devices=NCORE, num_swdge_queues=4,
                   dynamic_dma_scratch_size=32768)
    f32, bf16, i16 = mybir.dt.float32, mybir.dt.bfloat16, mybir.dt.int16
    AL = mybir.AluOpType
    TOT_SUB = nchunk * SUB
    FD = [HID, HID2, K]
    NT = NOWN // 128
    AG_GROUPS = [[0, 1], [2, 3], [4, 5], [6, 7]]
    RS_GROUPS = [[0, 2, 4, 6], [1, 3, 5, 7]]

    t_x = nc.dram_tensor("x", [NOWN, IN_DIM], f32, kind="ExternalInput")
    t_idx = nc.dram_tensor("idx", [nchunk, 128, CHUNK // 16], i16,
                           kind="ExternalInput")
    t_msk = nc.dram_tensor("msk", [nchunk, 128, SUB, 128], bf16,
                           kind="ExternalInput")
    t_W0 = nc.dram_tensor("W0p", [2, 128, HID], f32, kind="ExternalInput")
    t_P0 = nc.dram_tensor("P0p", [2, 128, HID], f32, kind="ExternalInput")
    t_W1 = nc.dram_tensor("W1p", [1, 128, HID2], f32, kind="ExternalInput")
    t_P1 = nc.dram_tensor("P1p", [1, 128, HID2], f32, kind="ExternalInput")
    t_W2 = nc.dram_tensor("W2p", [1, 128, K], f32, kind="ExternalInput")
    t_b0 = nc.dram_tensor("b0r", [128, HID], f32, kind="ExternalInput")
    t_b1 = nc.dram_tensor("b1r", [128, HID2], f32, kind="ExternalInput")
    t_b2 = nc.dram_tensor("b2r", [128, K], f32, kind="ExternalInput")
    t_dd = nc.dram_tensor("ddr", [128, NT], f32, kind="ExternalInput")
    t_id = nc.dram_tensor("ident", [128, 128], f32, kind="ExternalInput")
    t_out = nc.dram_tensor("out", [NOWN, K], f32, kind="ExternalOutput")

    with tile.TileContext(nc) as tc:
        with (
            tc.tile_pool(name="dram", bufs=1, space="DRAM") as dpool,
            tc.tile_pool(name="wts", bufs=1) as wpool,
            tc.tile_pool(name="gbuf", bufs=5) as gpool,
            tc.tile_pool(name="mbuf", bufs=4) as mpool,
            tc.tile_pool(name="work", bufs=4) as pool,
            tc.tile_pool(name="ps", bufs=2, space="PSUM") as ppool,
            tc.tile_pool(name="acc", bufs=2, space="PSUM") as apool,
        ):
            d_tab_own = [[dpool.tile([NH[h], 128], bf16,
                                     tag=f"tabown{l}_{h}",
                                     name=f"tabown{l}_{h}")
                          for h in range(2)] for l in range(3)]
            d_tab_pair = [dpool.tile([NPAIR, 128], bf16, tag=f"tabpair{l}",
                                     name=f"tabpair{l}") for l in range(3)]
            d_sk = [[dpool.tile([NK[k] , FD[l]], f32, tag=f"sk{l}_{k}",
                                name=f"sk{l}_{k}") for k in range(4)]
                    for l in range(3)]
            d_part = [[dpool.tile([4 * NK[k], FD[l]], bf16,
                                  tag=f"part{l}_{k}", name=f"part{l}_{k}")
                       for k in range(4)] for l in range(3)]
            d_rs = [[dpool.tile([NK[k], FD[l]], bf16, tag=f"rs{l}_{k}",
                                name=f"rs{l}_{k}") for k in range(4)]
                    for l in range(3)]
            d_h = [[dpool.tile([NK[k], FD[l]], f32, tag=f"h{l}_{k}",
                               name=f"h{l}_{k}") for k in range(4)]
                   for l in range(2)]

            idt = wpool.tile([128, 128], f32)
            nc.sync.dma_start(idt[:], t_id[:])
            def wload(tname, src_t, n, fdim):
                ts = []
                for j in range(n):
                    wt = wpool.tile([128, fdim], f32, name=f"{tname}_{j}")
                    nc.sync.dma_start(wt[:], src_t[j])
                    ts.append(wt)
                return ts
            wW0 = wload("wW0", t_W0, 2, HID)
            wP0 = wload("wP0", t_P0, 2, HID)
            wW1 = wload("wW1", t_W1, 1, HID2)
            wP1 = wload("wP1", t_P1, 1, HID2)
            wW2 = wload("wW2", t_W2, 1, K)
            wb0 = wpool.tile([128, HID], f32)
            nc.sync.dma_start(wb0[:], t_b0[:])
            wb1 = wpool.tile([128, HID2], f32)
            nc.sync.dma_start(wb1[:], t_b1[:])
            wb2 = wpool.tile([128, K], f32)
            nc.sync.dma_start(wb2[:], t_b2[:])
            wdd = wpool.tile([128, NT], f32)
            nc.sync.dma_start(wdd[:], t_dd[:])

            def transform_tile(l, t, src_ap, fin, fout, Wt, Pt, bias):
                """One 128-row tile: table entry + skip(+self) entry."""
                ncin = (fin + 127) // 128
                k = min(t // 25, 3)
                h = 0 if t < 50 else 1
                xin = pool.tile([128, ncin * 128], f32, tag="xin")
                if fin % 128:
                    nc.vector.memzero(xin[:, fin:])
                nc.sync.dma_start(xin[:, :fin], src_ap)
                xT = pool.tile([128, ncin * 128], f32, tag="xT")
                for j in range(ncin):
                    pt = ppool.tile([128, 128], f32, tag="ptr")
                    nc.tensor.transpose(
                        pt[:], xin[:, j * 128:(j + 1) * 128], idt[:])
                    nc.vector.tensor_copy(
                        xT[:, j * 128:(j + 1) * 128], pt[:])
                pm = ppool.tile([128, fout], f32, tag="pmm")
                for j in range(ncin):
                    nc.tensor.matmul(pm[:], xT[:, j * 128:(j + 1) * 128],
                                     Wt[j][:], start=(j == 0),
                                     stop=(j == ncin - 1))
                tb = pool.tile([128, 128], bf16, tag="tabtile")
                nc.vector.tensor_copy(tb[:, :fout], pm[:])
                nc.scalar.dma_start(
                    d_tab_own[l][h][t * 128 - 6400 * h:
                                    t * 128 - 6400 * h + 128, :], tb[:])
                # skip path, with the self-loop message dd*(h@W) folded in
                sk = pool.tile([128, fout], f32, tag="sktile")
                nc.vector.tensor_scalar(sk[:], pm[:], wdd[:, t:t + 1], None,
                                        AL.mult)
                if Pt is not None:
                    ps = ppool.tile([128, fout], f32, tag="psk")
                    for j in range(ncin):
                        nc.tensor.matmul(ps[:], xT[:, j * 128:(j + 1) * 128],
                                         Pt[j][:], start=(j == 0),
                                         stop=(j == ncin - 1))
                    nc.vector.tensor_tensor(sk[:], sk[:], ps[:], AL.add)
                nc.vector.tensor_tensor(sk[:], sk[:], bias[:, :fout], AL.add)
                nc.scalar.dma_start(
                    d_sk[l][k][t * 128 - 3200 * k:
                               t * 128 - 3200 * k + 128, :], sk[:])

            def allgather(l, h):
                nc.gpsimd.collective_compute(
                    "AllGather", mybir.AluOpType.bypass,
                    replica_groups=AG_GROUPS,
                    ins=[d_tab_own[l][h][:].opt()],
                    outs=[d_tab_pair[l][HB[h]:HB[h] + 2 * NH[h], :].opt()])

            def aggregate(l, fout):
                acc = None
                sub = 0
                rs_pending = []
                last_ev = None
                def emit_rs(k):
                    nc.gpsimd.collective_compute(
                        "ReduceScatter", mybir.AluOpType.add,
                        replica_groups=RS_GROUPS,
                        ins=[d_part[l][k][:].opt()],
                        outs=[d_rs[l][k][:].opt()])
                for ch in range(nchunk):
                    while rs_pending and ch >= rs_pending[0][1]:
                        emit_rs(rs_pending.pop(0)[0])
                    it = pool.tile([128, CHUNK // 16], i16, tag="idxt",
                                   bufs=5)
                    nc.sync.dma_start(it[:], t_idx[ch])
                    g = gpool.tile([128, SUB, 128], bf16, tag="gath")
                    nc.gpsimd.dma_gather(g[:], d_tab_pair[l][:], it[:],
                                         CHUNK, CHUNK, 128,
                                         single_packet=False,
                                         queue_num=ch % 4)
                    mk = mpool.tile([128, SUB, 128], bf16, tag="maskt")
                    nc.sync.dma_start(mk[:], t_msk[ch])
                    for j in range(SUB):
                        st = bool(start_of_sub[sub])
                        if st:
                            acc = apool.tile([128, fout], f32, tag="accps",
                                             name=f"acc{l}_{sub}")
                        last = (sub == TOT_SUB - 1) or bool(
                            start_of_sub[sub + 1])
                        nc.tensor.matmul(acc[:], mk[:, j, :],
                                         g[:, j, :fout], start=st, stop=last)
                        if last:
                            blk = int(blk_of_sub[sub])
                            k = min(blk // 100, 3)
                            lofs = (blk - RBLK_EDGE[k]) * 128
                            ev = pool.tile([128, fout], bf16, tag="ev", bufs=8)
                            nc.vector.tensor_copy(ev[:], acc[:])
                            nc.scalar.dma_start(
                                d_part[l][k][lofs:lofs + 128, :], ev[:])
                            last_ev = ev
                            if blk == RBLK_EDGE[k + 1] - 1:
                                rs_pending.append((k, ch + 3))
                        sub += 1
                for k, _ in rs_pending:
                    emit_rs(k)
                # zero-valued gate produced by the final eviction: combine's
                # first op reads it, so neither the Tile scheduler nor the
                # hardware can slot combine work (and its queue-head waits)
                # into the middle of the gather-bound aggregation streams
                gate = pool.tile([128, 1], f32, tag="gate", bufs=2)
                nc.vector.tensor_scalar(gate[:], last_ev[:, 0:1], 0.0, None,
                                        AL.mult)
                return gate

            def combine(l, fout, k, gate):
                for tt in range(NTIL[k]):
                    zb = pool.tile([128, fout], bf16, tag="zb", bufs=8)
                    nc.sync.dma_start(zb[:],
                                      d_rs[l][k][tt * 128:(tt + 1) * 128, :])
                    sk = pool.tile([128, fout], f32, tag="skld", bufs=8)
                    nc.sync.dma_start(sk[:],
                                      d_sk[l][k][tt * 128:(tt + 1) * 128, :])
                    zz = pool.tile([128, fout], f32, tag="z", bufs=8)
                    nc.vector.tensor_scalar(zz[:], zb[:], gate[:], None,
                                            AL.add)
                    nc.vector.tensor_tensor(zz[:], zz[:], sk[:], AL.add)
                    mn = pool.tile([128, fout], f32, tag="smn", bufs=8)
                    nc.vector.tensor_scalar_min(mn[:], zz[:], 0.0)
                    ex = pool.tile([128, fout], f32, tag="sex", bufs=8)
                    nc.scalar.activation(ex[:], mn[:],
                                         mybir.ActivationFunctionType.Exp)
                    nc.vector.tensor_scalar(ex[:], ex[:], SELU_L * SELU_A,
                                            -SELU_L * SELU_A, AL.mult,
                                            AL.add)
                    nc.vector.tensor_scalar_max(zz[:], zz[:], 0.0)
                    nc.vector.tensor_scalar(zz[:], zz[:], SELU_L, None,
                                            AL.mult)
                    nc.vector.tensor_tensor(zz[:], zz[:], ex[:], AL.add)
                    if l < 2:
                        nc.scalar.dma_start(
                            d_h[l][k][tt * 128:(tt + 1) * 128, :], zz[:])
                    else:
                        mx = pool.tile([128, 1], f32, tag="smx")
                        nc.vector.tensor_reduce(mx[:], zz[:],
                                                mybir.AxisListType.X, AL.max)
                        nc.vector.tensor_scalar(zz[:], zz[:], mx[:], None,
                                                AL.subtract)
                        nc.scalar.activation(
                            zz[:], zz[:], mybir.ActivationFunctionType.Exp)
                        sm = pool.tile([128, 1], f32, tag="ssm")
                        nc.vector.tensor_reduce(sm[:], zz[:],
                                                mybir.AxisListType.X, AL.add)
                        rc = pool.tile([128, 1], f32, tag="src")
                        nc.vector.reciprocal(rc[:], sm[:])
                        nc.vector.tensor_scalar(zz[:], zz[:], rc[:], None,
                                                AL.mult)
                        gofs = K_START[k] + tt * 128
                        nc.scalar.dma_start(t_out[gofs:gofs + 128, :], zz[:])

            fins = [IN_DIM, HID, HID2]
            Ws = [wW0, wW1, wW2]
            Ps = [wP0, wP1, None]
            bs = [wb0, wb1, wb2]

            # layer-0 transform over all tiles, AGs pipelined per half
            for t in range(NT):
                transform_tile(0, t, t_x[t * 128:(t + 1) * 128, :],
                               fins[0], FD[0], Ws[0], Ps[0], bs[0])
                if t == 49:
                    allgather(0, 0)
            allgather(0, 1)

            for l in range(3):
                gate = aggregate(l, FD[l])
                for k in range(4):
                    combine(l, FD[l], k, gate)
                    if l < 2:
                        l2 = l + 1
                        for t in range(25 * k, 25 * k + NTIL[k]):
                            src = d_h[l][k][t * 128 - 3200 * k:
                                            t * 128 - 3200 * k + 128, :]
                            transform_tile(l2, t, src, fins[l2], FD[l2],
                                           Ws[l2], Ps[l2], bs[l2])
                        if k == 1:
                            allgather(l2, 0)
                        if k == 3:
                            allgather(l2, 1)
    nc.compile()
    return nc


def _get_compiled(inputs):
    k = "prog"
    if k not in _CACHE:
        plan = _build_plan(inputs["edge_index"], inputs["edge_weight"])
        nc = _build_program(plan["nchunk"], plan["blk_of_sub"],
                            plan["start_of_sub"])
        _CACHE[k] = (plan, nc)
    return _CACHE[k]


def kernel(_trace=False, **inputs):
    from concourse.bass_utils import run_bass_kernel_spmd

    plan, nc = _get_compiled(inputs)

    x = np.asarray(inputs["x"], np.float32)
    xpad = np.zeros((NCORE, NOWN, IN_DIM), np.float32)
    for c in range(NCORE):
        xpad[c, :12500] = x[c * 12500:(c + 1) * 12500]

    def wchunks(W, n):
        out = np.zeros((n, 128, W.shape[1]), np.float32)
        for j in range(n):
            out[j, :min(128, W.shape[0] - j * 128)] = \
                W[j * 128:(j + 1) * 128]
        return out

    W0 = np.asarray(inputs["W0"], np.float32)
    P0 = np.asarray(inputs["P0w"], np.float32)
    W1p = wchunks(np.asarray(inputs["W1"], np.float32), 1)
    P1p = wchunks(np.asarray(inputs["P1w"], np.float32), 1)
    W2p = wchunks(np.asarray(inputs["W2"], np.float32), 1)
    b0r = np.tile((np.asarray(inputs["b0"]) + np.asarray(inputs["P0b"]))
                  .astype(np.float32), (128, 1))
    b1r = np.tile((np.asarray(inputs["b1"]) + np.asarray(inputs["P1b"]))
                  .astype(np.float32), (128, 1))
    b2r = np.tile(np.asarray(inputs["b2"]).astype(np.float32), (128, 1))

    in_maps = []
    for c in range(NCORE):
        in_maps.append({
            "x": xpad[c],
            "idx": plan["idx"][c], "msk": plan["msk"][c],
            "W0p": wchunks(W0, 2), "P0p": wchunks(P0, 2),
            "W1p": W1p, "P1p": P1p, "W2p": W2p,
            "b0r": b0r, "b1r": b1r, "b2r": b2r,
            "ddr": plan["ddp"][c],
            "ident": np.eye(128, dtype=np.float32),
        })
    res = run_bass_kernel_spmd(nc, in_maps, core_ids=list(range(NCORE)),
                               trace=_trace)
    if _trace:
        kernel.last_exec_ns = res.exec_time_ns
    out = np.zeros((N, K), np.float32)
    for c in range(NCORE):
        out[c * 12500:(c + 1) * 12500] = res.results[c]["out"][:12500]
    return out
